# revision 6
# baseline (speedup 1.0000x reference)
"""Causal multi-head attention with RoPE on 8 Trainium2 NeuronCores.

Sharding: core = (batch b, head-group hg): b = core//4, hg = core%4.
Each core computes 4 heads of one batch element end-to-end (QKV projection,
RoPE, causal softmax attention, output-projection partial) and the host sums
the 4 per-head-group partials per batch (the "all-reduce" of the O-proj).

v2 changes vs baseline (252us):
  - causal mask applied on the PE as a matmul accumulate (-1e5 upper-tri
    matrix times identity into the score psum before exp) instead of a DVE
    multiply in the score->exp->AV critical chain.
  - input DMAs split across three queues (SP: xT, ACT-hwdge: weights,
    gpsimd: tables) in first-use order; cos/sin tables halved (64 rows +
    on-device duplicate); vones/ones inputs replaced by memsets.
  - warmup matmul chain + early dummy exp to hold the PE HAM clock at
    2.4 GHz through the initial DMA fill (cold PE runs at 1.2 GHz).
  - filler-thunk scheduler: next-chunk projections + prev-chunk norm and
    output-projection are interleaved into each attention phase so the PE
    stream stays dense while ACT runs the exps.
  - per-head softmax-denominator dance (reciprocal pipelined per head),
    batched y stores ([128,1024] per l-tile).
Score-path matmuls run in float32r (TF32); the P*V path runs in bf16.
"""

import numpy as np

_B, _L, _D, _H, _HD = 2, 2048, 1024, 16, 64
_HPG = 4              # heads per group (per core)
_EG = _HPG * _HD      # 256
_NCORES = 8
_THETA = 10000.0
_QC = 512             # q-chunk width
_NQC = _L // _QC      # 4
_GK = 2               # k-tiles (128) per exp group
_NKC = _D // 128      # 8 contraction chunks for projections
_LC = 512             # l-chunk
_NWARM = 24           # PE warmup matmuls (256-col each)

_CACHE = {}


def _tf32(a):
    """Round float32 array to TF32 (fp32r): RNE to 10-bit mantissa."""
    b = np.ascontiguousarray(a, dtype=np.float32).view(np.uint32)
    b = (b + np.uint32(0xFFF) + ((b >> np.uint32(13)) & np.uint32(1))) \
        & np.uint32(0xFFFFE000)
    return b.view(np.float32)


def _build_nc():
    from contextlib import ExitStack

    import concourse.mybir as mybir
    import concourse.tile as tile
    from concourse import bacc

    f32 = mybir.dt.float32
    f32r = mybir.dt.float32r
    bf16 = mybir.dt.bfloat16
    EXP = mybir.ActivationFunctionType.Exp

    nc = bacc.Bacc("TRN2", target_bir_lowering=False, debug=False,
                   enable_asserts=False)
    xT = nc.dram_tensor("xT", [_D, _L], f32r, kind="ExternalInput")
    wq = nc.dram_tensor("wq", [_D, _EG], f32r, kind="ExternalInput")
    wk = nc.dram_tensor("wk", [_D, _EG], f32r, kind="ExternalInput")
    wv = nc.dram_tensor("wv", [_D, _EG], f32r, kind="ExternalInput")
    wo = nc.dram_tensor("wo", [_EG, _D], f32r, kind="ExternalInput")
    cs2 = nc.dram_tensor("cs2", [64, _L], f32, kind="ExternalInput")
    sn2 = nc.dram_tensor("sn2", [64, _L], f32, kind="ExternalInput")
    perm = nc.dram_tensor("perm", [128, 128], f32r, kind="ExternalInput")
    maskM = nc.dram_tensor("maskM", [128, 128], f32r, kind="ExternalInput")
    id128 = nc.dram_tensor("id128", [128, 128], f32r, kind="ExternalInput")
    y = nc.dram_tensor("y", [_L, _D], f32, kind="ExternalOutput")

    with tile.TileContext(nc) as tc, ExitStack() as ctx:
        persist = ctx.enter_context(tc.tile_pool(name="persist", bufs=1))
        qT_sb = persist.tile([128, 2, _L], f32r)
        kT_sb = persist.tile([128, 2, _L], f32r)
        v_sb = persist.tile([128, _L // 128, _HPG, _HD + 4], bf16)
        oT_sb = persist.tile([128, 2, _L], f32r)
        wo_sb = persist.tile([128, 2, _D], f32r)
        wq_sb = persist.tile([128, _NKC, _EG], f32r)
        wk_sb = persist.tile([128, _NKC, _EG], f32r)
        wv_sb = persist.tile([128, _NKC, _EG], f32r)
        cs_sb = persist.tile([128, _L], f32)
        sn_sb = persist.tile([128, _L], f32)
        perm_sb = persist.tile([128, 128], f32r)
        mask_sb = persist.tile([128, 128], f32r)
        id_sb = persist.tile([128, 128], f32r)
        ones_sb = persist.tile([65, 64], f32r)
        warm_sb = persist.tile([128, 256], f32r)

        xtp = ctx.enter_context(tc.tile_pool(name="xtp", bufs=3))
        rtmp = ctx.enter_context(tc.tile_pool(name="rtmp", bufs=2))
        ptp = ctx.enter_context(tc.tile_pool(name="ptp", bufs=4))
        nrm = ctx.enter_context(tc.tile_pool(name="nrm", bufs=3))
        otcp = ctx.enter_context(tc.tile_pool(name="otc", bufs=2))
        # PSUM budget (8 banks): sps 2x2 + ops 2x1 + scr 2x1
        sps = ctx.enter_context(tc.tile_pool(name="sps", bufs=2, space="PSUM"))
        ops = ctx.enter_context(tc.tile_pool(name="ops", bufs=2, space="PSUM"))
        scr = ctx.enter_context(tc.tile_pool(name="scr", bufs=2, space="PSUM"))

        # --- warmup: memsets + dummy exp (forces ACT table load) + PE
        # matmul chain so the HAM clock is at 8/8 when real work lands ---
        nc.vector.memset(warm_sb.bitcast(f32), 0.0)
        nc.vector.memset(ones_sb.bitcast(f32), 1.0)
        nc.vector.memset(v_sb[:, :, :, _HD].bitcast(mybir.dt.uint16), 0x3F80)
        wexp = ptp.tile([1, 16], bf16, tag="pt", name="wexp")
        nc.scalar.activation(wexp, warm_sb[0:1, 0:16], EXP, scale=0.125)
        for i in range(_NWARM):
            wp = scr.tile([128, 256], f32, tag="scr", name=f"warm{i}")
            nc.tensor.matmul(wp, warm_sb[:, 0:128], warm_sb,
                             start=True, stop=True)

        # --- input loads, split across queues in first-use order ---
        wq_r = wq.rearrange("(c p) e -> p c e", p=128)
        wk_r = wk.rearrange("(c p) e -> p c e", p=128)
        wv_r = wv.rearrange("(c p) e -> p c e", p=128)
        xT_r = xT.rearrange("(c p) l -> p c l", p=128)
        # gpsimd queue: small tables
        nc.gpsimd.dma_start(out=perm_sb, in_=perm[:, :])
        nc.gpsimd.dma_start(out=mask_sb, in_=maskM[:, :])
        nc.gpsimd.dma_start(out=id_sb, in_=id128[:, :])
        nc.gpsimd.dma_start(out=cs_sb[0:64, :], in_=cs2[:, :])
        nc.gpsimd.dma_start(out=sn_sb[0:64, :], in_=sn2[:, :])
        nc.gpsimd.dma_start(out=cs_sb[64:128, :], in_=cs_sb[0:64, :])
        nc.gpsimd.dma_start(out=sn_sb[64:128, :], in_=sn_sb[0:64, :])
        # scalar (ACT-hwdge) queue: weights (ACT idle at startup)
        for kc in range(_NKC):
            nc.scalar.dma_start(out=wq_sb[:, kc, :], in_=wq_r[:, kc, :])
            nc.scalar.dma_start(out=wk_sb[:, kc, :], in_=wk_r[:, kc, :])
        for kc in range(_NKC):
            nc.scalar.dma_start(out=wv_sb[:, kc, :], in_=wv_r[:, kc, :])
        nc.scalar.dma_start(out=wo_sb,
                            in_=wo.rearrange("(c p) d -> p c d", p=128))
        # sync (SP) queue: x chunks 0..2 up front (chunk 3 emitted later)
        xts = {}

        def load_xt(lc):
            xt = xtp.tile([128, _NKC, _LC], f32r, tag="xt", name=f"xt{lc}")
            for kc in range(_NKC):
                nc.sync.dma_start(
                    out=xt[:, kc, :],
                    in_=xT_r[:, kc, lc * _LC:(lc + 1) * _LC])
            xts[lc] = xt

        for lc in range(3):
            load_xt(lc)

        # --- work thunks ---
        def proj_qk_thunk(lc, w_sb, dst, c):
            def t():
                ls = slice(lc * _LC, (lc + 1) * _LC)
                xt = xts[lc]
                ps = scr.tile([128, _LC], f32, tag="scr",
                              name=f"ps{lc}_{c}")
                for kc in range(_NKC):
                    nc.tensor.matmul(
                        ps, w_sb[:, kc, c * 128:(c + 1) * 128],
                        xt[:, kc, :],
                        start=(kc == 0), stop=(kc == _NKC - 1))
                nc.vector.tensor_copy(dst[:, c, ls], ps)
            return t

        def rope_thunk(lc, dst, c):
            def t():
                ls = slice(lc * _LC, (lc + 1) * _LC)
                rp = scr.tile([128, _LC], f32, tag="scr",
                              name=f"rp{lc}_{c}")
                nc.tensor.matmul(rp, perm_sb[:, :], dst[:, c, ls],
                                 start=True, stop=True)
                tmp = rtmp.tile([128, _LC], f32, tag="rt")
                nc.vector.tensor_mul(tmp, rp, sn_sb[:, ls])
                nc.vector.tensor_mul(dst[:, c, ls], dst[:, c, ls],
                                     cs_sb[:, ls])
                nc.vector.tensor_add(dst[:, c, ls], dst[:, c, ls], tmp)
            return t

        def proj_v_thunk(lc, j):
            def t():
                xt = xts[lc]
                lt = lc * (_LC // 128) + j
                pv = scr.tile([128, _EG], f32, tag="scr", name=f"pv{lt}")
                for kc in range(_NKC):
                    nc.tensor.matmul(
                        pv, xt[:, kc, j * 128:(j + 1) * 128],
                        wv_sb[:, kc, :],
                        start=(kc == 0), stop=(kc == _NKC - 1))
                nc.vector.tensor_copy(
                    v_sb[:, lt, :, :_HD],
                    pv.rearrange("p (h e) -> p h e", h=_HPG))
            return t

        def norm_thunk(st, h):
            def t():
                qc, qs = st["qc"], st["qs"]
                c, pb = h // 2, 64 * (h % 2)
                drow, otc = st["drow"][h], st["otcs"][h]
                # rank-1 broadcast: ones[1,64].T @ recip_row -> [64, 512]
                bc = scr.tile([128, _QC], f32, tag="scr",
                              name=f"bc{qc}_{h}")
                nc.tensor.matmul(
                    bc[0:64, :], ones_sb[64:65, :], drow[64:65, :],
                    start=True, stop=True)
                otn = otcp.tile([64, _QC], f32r, tag="otn", bufs=3,
                                name=f"otn{qc}_{h}")
                nc.vector.tensor_mul(otn, otc[0:64, :], bc[0:64, :])
                # place normalized O^T at this head's partitions (DMA can
                # cross partition bases; compute engines cannot)
                nc.gpsimd.dma_start(out=oT_sb[pb:pb + 64, c, qs], in_=otn)
            return t

        def oproj_thunk(st, j):
            def t():
                qc = st["qc"]
                lt = qc * (_QC // 128) + j
                ob = otcp.tile([128, _D], f32, tag="ob", bufs=2,
                               name=f"ob{qc}_{j}")
                for n in range(2):
                    op = scr.tile([128, 512], f32, tag="scr",
                                  name=f"op{qc}_{j}_{n}")
                    for cc in range(2):
                        nc.tensor.matmul(
                            op, oT_sb[:, cc, lt * 128:(lt + 1) * 128],
                            wo_sb[:, cc, n * 512:(n + 1) * 512],
                            start=(cc == 0), stop=(cc == 1))
                    nc.vector.tensor_copy(ob[:, n * 512:(n + 1) * 512], op)
                nc.sync.dma_start(
                    out=y[lt * 128:(lt + 1) * 128, :], in_=ob)
            return t

        # --- filler machinery ---
        state = {"fillers": [], "fi": 0}

        def pop_filler(n=1):
            for _ in range(n):
                if state["fi"] < len(state["fillers"]):
                    state["fillers"][state["fi"]]()
                    state["fi"] += 1

        def drain_fillers():
            pop_filler(len(state["fillers"]) - state["fi"])

        # chunk 0 projections emitted directly
        for c in range(2):
            proj_qk_thunk(0, wq_sb, qT_sb, c)()
            proj_qk_thunk(0, wk_sb, kT_sb, c)()
        for dst in (qT_sb, kT_sb):
            for c in range(2):
                rope_thunk(0, dst, c)()
        for j in range(_LC // 128):
            proj_v_thunk(0, j)()

        prev = None
        for qc in range(_NQC):
            drain_fillers()
            # build filler list: prev-chunk norm, next-chunk projections,
            # prev-chunk output projection (emission order = PE order)
            fl = []
            if prev is not None:
                for h in range(_HPG):
                    fl.append(norm_thunk(prev, h))
            if qc + 1 < _NQC:
                if qc == 1:
                    load_xt(3)
                for c in range(2):
                    fl.append(proj_qk_thunk(qc + 1, wq_sb, qT_sb, c))
                    fl.append(proj_qk_thunk(qc + 1, wk_sb, kT_sb, c))
                for dst in (qT_sb, kT_sb):
                    for c in range(2):
                        fl.append(rope_thunk(qc + 1, dst, c))
                for j in range(_LC // 128):
                    fl.append(proj_v_thunk(qc + 1, j))
            if prev is not None:
                for j in range(_QC // 128):
                    fl.append(oproj_thunk(prev, j))
            state["fillers"] = fl
            state["fi"] = 0

            # ---- attention for q-chunk qc ----
            q0 = qc * _QC
            qs = slice(q0, q0 + _QC)
            nkt = (qc + 1) * (_QC // 128)
            otcs = []
            drows = []
            for h in range(_HPG):
                c, pb = h // 2, 64 * (h % 2)
                ot = ops.tile([_HD + 1, _QC], f32, tag="ot")
                ngr = (nkt + _GK - 1) // _GK
                for g in range(ngr):
                    kts = list(range(g * _GK, min((g + 1) * _GK, nkt)))
                    sp = sps.tile([128, _GK * _QC], f32, tag="sp")
                    # q columns < dj*128 of a diagonal k-tile are entirely
                    # in the causal-masked region: skip them in scores,
                    # exp and AV (triangular decomposition)
                    for i, kt in enumerate(kts):
                        dj = kt - qc * (_QC // 128)
                        lo = max(dj, 0) * 128
                        nc.tensor.matmul(
                            sp[:, i * _QC + lo:(i + 1) * _QC],
                            kT_sb[pb:pb + 64, c, kt * 128:(kt + 1) * 128],
                            qT_sb[pb:pb + 64, c, q0 + lo:q0 + _QC],
                            start=True, stop=(dj < 0),
                            skip_group_check=True)
                        if dj >= 0:
                            # causal mask: accumulate -1e5 upper-tri into
                            # the diagonal 128-col slice of this k-tile
                            nc.tensor.matmul(
                                sp[:, i * _QC + lo:i * _QC + lo + 128],
                                mask_sb[:, :], id_sb[:, :],
                                start=False, stop=True,
                                skip_group_check=True)
                    pt = ptp.tile([128, _GK * _QC], bf16, tag="pt")
                    diag = any(kt - qc * (_QC // 128) >= 0 for kt in kts)
                    if not diag:
                        na = len(kts) * _QC
                        nc.scalar.activation(pt[:, :na], sp[:, :na], EXP,
                                             scale=0.125)
                    else:
                        # ragged tile starts: exp per tile's written span
                        for i, kt in enumerate(kts):
                            lo = max(kt - qc * (_QC // 128), 0) * 128
                            nc.scalar.activation(
                                pt[:, i * _QC + lo:(i + 1) * _QC],
                                sp[:, i * _QC + lo:(i + 1) * _QC], EXP,
                                scale=0.125)
                    for i, kt in enumerate(kts):
                        lo = max(kt - qc * (_QC // 128), 0) * 128
                        nc.tensor.matmul(
                            ot[:, lo:], v_sb[:, kt, h, :_HD + 1],
                            pt[:, i * _QC + lo:(i + 1) * _QC],
                            start=(kt == 0), stop=(kt == nkt - 1),
                            skip_group_check=True)
                    pop_filler()
                # per-head normalization dance: copy numerator to SBUF,
                # stash the denominator row transposed to [128, 4] so the
                # reciprocal is partition-parallel, restore to a row
                otc = otcp.tile([_HD + 1, _QC], f32, tag="otc", bufs=6,
                                name=f"otc{qc}_{h}")
                nc.scalar.copy(otc, ot[:, :])
                dsb = nrm.tile([128, 4], f32, tag="dsb")
                nc.gpsimd.dma_start(out=dsb, in_=otc[64:65, :])
                drec = nrm.tile([128, 4], f32, tag="drec")
                nc.vector.reciprocal(drec, dsb)
                drecr = nrm.tile([128, 4], f32r, tag="drecr")
                nc.vector.tensor_copy(drecr, drec)
                drow = nrm.tile([65, _QC], f32r, tag="drow", bufs=6,
                                name=f"drow{qc}_{h}")
                nc.gpsimd.dma_start(out=drow[64:65, :], in_=drecr)
                otcs.append(otc)
                drows.append(drow)
                pop_filler()
            prev = {"qc": qc, "qs": qs, "otcs": otcs, "drow": drows}
        # tail: last chunk's normalization + output projection
        drain_fillers()
        for h in range(_HPG):
            norm_thunk(prev, h)()
        for j in range(_QC // 128):
            oproj_thunk(prev, j)()
    nc.compile()
    return nc


def get_nc():
    if "nc" not in _CACHE:
        _CACHE["nc"] = _build_nc()
    return _CACHE["nc"]


def make_in_maps(x, token_positions, Q, K, V, O_w):
    """Host-side sharding: per-core input dict (core = b*4 + hg)."""
    x = np.asarray(x, dtype=np.float32)
    tp = np.asarray(token_positions)
    Q = np.asarray(Q, dtype=np.float32)
    K = np.asarray(K, dtype=np.float32)
    V = np.asarray(V, dtype=np.float32)
    O_w = np.asarray(O_w, dtype=np.float32)

    # RoPE tables, [64, L]: rows = head-local e (cos/sin repeated pairwise);
    # the device duplicates to partitions 64..127 (two heads per tile).
    i = np.arange(_HD // 2, dtype=np.float64)
    denom = _THETA ** (2.0 * i / _HD)                      # [32]
    ang = tp.astype(np.float64)[None, :] / denom[:, None]  # [32, L]
    cs2 = np.repeat(np.cos(ang), 2, axis=0).astype(np.float32)
    sn2 = np.repeat(np.sin(ang), 2, axis=0).astype(np.float32)

    # pairwise-rotation permutation (rot(x)[2i] = -x[2i+1], rot[2i+1] = x[2i])
    # as a stationary operand: out = permT.T @ x^T = Perm @ x^T
    p64 = np.zeros((64, 64), np.float32)
    for j in range(_HD // 2):
        p64[2 * j + 1, 2 * j] = -1.0
        p64[2 * j, 2 * j + 1] = 1.0
    permT = np.zeros((128, 128), np.float32)
    permT[0:64, 0:64] = p64
    permT[64:128, 64:128] = p64

    # causal mask as an additive stationary operand: matmul adds
    # maskM.T (-1e5 where q' < k) into the diagonal score tile
    a = np.arange(128)
    maskM = np.where(a[:, None] < a[None, :], -1.0e5, 0.0).astype(np.float32)
    id128 = np.eye(128, dtype=np.float32)

    Qr = Q.reshape(_H, _HD, _D)
    Kr = K.reshape(_H, _HD, _D)
    Vr = V.reshape(_H, _HD, _D)

    in_maps = []
    xT = [_tf32(x[b].T) for b in range(_B)]
    for core in range(_NCORES):
        b, hg = core // 4, core % 4
        hs = slice(hg * _HPG, (hg + 1) * _HPG)
        in_maps.append({
            "xT": xT[b],
            "wq": _tf32(Qr[hs].reshape(_EG, _D).T),
            "wk": _tf32(Kr[hs].reshape(_EG, _D).T),
            "wv": _tf32(Vr[hs].reshape(_EG, _D).T),
            "wo": _tf32(O_w[:, hg * _EG:(hg + 1) * _EG].T),
            "cs2": cs2, "sn2": sn2, "perm": permT,
            "maskM": _tf32(maskM), "id128": id128,
        })
    return in_maps


def run_on_hw(in_maps, trace=False, **kw):
    from concourse.bass_utils import run_bass_kernel_spmd
    nc = get_nc()
    return run_bass_kernel_spmd(nc, in_maps, core_ids=list(range(_NCORES)),
                                trace=trace, **kw)


def kernel(x, token_positions, Q, K, V, O_w):
    in_maps = make_in_maps(x, token_positions, Q, K, V, O_w)
    res = run_on_hw(in_maps)
    out = np.zeros((_B, _L, _D), dtype=np.float32)
    for core in range(_NCORES):
        out[core // 4] += res.results[core]["y"]
    return out


# revision 18
# speedup vs baseline: 1.1174x; 1.1174x over previous
"""Causal multi-head attention with RoPE on 8 Trainium2 NeuronCores.

Sharding: core = (batch b, head-group hg): b = core//4, hg = core%4.
Each core computes 4 heads of one batch element end-to-end (QKV projection,
RoPE, causal softmax attention, output-projection partial) and the host sums
the 4 per-head-group partials per batch (the "all-reduce" of the O-proj).

v2 changes vs baseline (252us):
  - causal mask applied on the PE as a matmul accumulate (-1e5 upper-tri
    matrix times identity into the score psum before exp) instead of a DVE
    multiply in the score->exp->AV critical chain.
  - input DMAs split across three queues (SP: xT, ACT-hwdge: weights,
    gpsimd: tables) in first-use order; cos/sin tables halved (64 rows +
    on-device duplicate); vones/ones inputs replaced by memsets.
  - warmup matmul chain + early dummy exp to hold the PE HAM clock at
    2.4 GHz through the initial DMA fill (cold PE runs at 1.2 GHz).
  - filler-thunk scheduler: next-chunk projections + prev-chunk norm and
    output-projection are interleaved into each attention phase so the PE
    stream stays dense while ACT runs the exps.
  - per-head softmax-denominator dance (reciprocal pipelined per head),
    batched y stores ([128,1024] per l-tile).
Score-path matmuls run in float32r (TF32); the P*V path runs in bf16.
"""

import numpy as np

_B, _L, _D, _H, _HD = 2, 2048, 1024, 16, 64
_HPG = 4              # heads per group (per core)
_EG = _HPG * _HD      # 256
_NCORES = 8
_THETA = 10000.0
_QC = 512             # q-chunk width
_NQC = _L // _QC      # 4
_GK = 2               # k-tiles (128) per exp group
_NKC = _D // 128      # 8 contraction chunks for projections
_LC = 512             # l-chunk
_NWARM = 24           # PE warmup matmuls (256-col each)

_CACHE = {}


def _tf32(a):
    """Round float32 array to TF32 (fp32r): RNE to 10-bit mantissa."""
    b = np.ascontiguousarray(a, dtype=np.float32).view(np.uint32)
    b = (b + np.uint32(0xFFF) + ((b >> np.uint32(13)) & np.uint32(1))) \
        & np.uint32(0xFFFFE000)
    return b.view(np.float32)


def _build_nc():
    from contextlib import ExitStack

    import concourse.mybir as mybir
    import concourse.tile as tile
    from concourse import bacc

    f32 = mybir.dt.float32
    f32r = mybir.dt.float32r
    bf16 = mybir.dt.bfloat16
    EXP = mybir.ActivationFunctionType.Exp

    nc = bacc.Bacc("TRN2", target_bir_lowering=False, debug=False,
                   enable_asserts=False)
    xT = nc.dram_tensor("xT", [_D, _L], bf16, kind="ExternalInput")
    wq = nc.dram_tensor("wq", [_D, _EG], bf16, kind="ExternalInput")
    wk = nc.dram_tensor("wk", [_D, _EG], bf16, kind="ExternalInput")
    wv = nc.dram_tensor("wv", [_D, _EG], bf16, kind="ExternalInput")
    wo = nc.dram_tensor("wo", [_EG, _D], bf16, kind="ExternalInput")
    cs2 = nc.dram_tensor("cs2", [64, _L], f32, kind="ExternalInput")
    sn2 = nc.dram_tensor("sn2", [64, _L], f32, kind="ExternalInput")
    perm = nc.dram_tensor("perm", [128, 128], f32r, kind="ExternalInput")
    maskM = nc.dram_tensor("maskM", [128, 128], f32r, kind="ExternalInput")
    id128 = nc.dram_tensor("id128", [128, 128], f32r, kind="ExternalInput")
    y = nc.dram_tensor("y", [_L, _D], f32, kind="ExternalOutput")

    with tile.TileContext(nc) as tc, ExitStack() as ctx:
        persist = ctx.enter_context(tc.tile_pool(name="persist", bufs=1))
        qT_sb = persist.tile([128, 2, _L], f32r)
        kT_sb = persist.tile([128, 2, _L], f32r)
        v_sb = persist.tile([128, _L // 128, _HPG, _HD + 4], bf16)
        oT_sb = persist.tile([128, 2, _L], bf16)
        wo_sb = persist.tile([128, 2, _D], bf16)
        wq_sb = persist.tile([128, _NKC, _EG], bf16)
        wk_sb = persist.tile([128, _NKC, _EG], bf16)
        wv_sb = persist.tile([128, _NKC, _EG], bf16)
        cs_sb = persist.tile([128, _L], f32)
        sn_sb = persist.tile([128, _L], f32)
        perm_sb = persist.tile([128, 128], f32r)
        mask_sb = persist.tile([128, 128], f32r)
        id_sb = persist.tile([128, 128], f32r)
        ones_sb = persist.tile([65, 64], f32r)
        warm_sb = persist.tile([128, 256], f32r)

        xtp = ctx.enter_context(tc.tile_pool(name="xtp", bufs=4))
        rtmp = ctx.enter_context(tc.tile_pool(name="rtmp", bufs=2))
        ptp = ctx.enter_context(tc.tile_pool(name="ptp", bufs=4))
        nrm = ctx.enter_context(tc.tile_pool(name="nrm", bufs=3))
        otcp = ctx.enter_context(tc.tile_pool(name="otc", bufs=2))
        # PSUM budget (8 banks): sps 2x2 + ops 2x1 + scr 2x1
        sps = ctx.enter_context(tc.tile_pool(name="sps", bufs=2, space="PSUM"))
        ops = ctx.enter_context(tc.tile_pool(name="ops", bufs=2, space="PSUM"))
        scr = ctx.enter_context(tc.tile_pool(name="scr", bufs=2, space="PSUM"))

        # --- warmup: memsets + dummy exp (forces ACT table load) + PE
        # matmul chain so the HAM clock is at 8/8 when real work lands ---
        nc.vector.memset(warm_sb.bitcast(f32), 0.0)
        nc.vector.memset(ones_sb.bitcast(f32), 1.0)
        nc.vector.memset(v_sb[:, :, :, _HD].bitcast(mybir.dt.uint16), 0x3F80)
        wexp = ptp.tile([1, 16], bf16, tag="pt", name="wexp")
        nc.scalar.activation(wexp, warm_sb[0:1, 0:16], EXP, scale=0.125)
        for i in range(_NWARM):
            wp = scr.tile([128, 256], f32, tag="scr", name=f"warm{i}")
            nc.tensor.matmul(wp, warm_sb[:, 0:128], warm_sb,
                             start=True, stop=True)

        # --- input loads, split across queues in first-use order ---
        wq_r = wq.rearrange("(c p) e -> p c e", p=128)
        wk_r = wk.rearrange("(c p) e -> p c e", p=128)
        wv_r = wv.rearrange("(c p) e -> p c e", p=128)
        xT_r = xT.rearrange("(c p) l -> p c l", p=128)
        # three queues stream in parallel, ordered by first use:
        #   sync (SP-hwdge):    xT chunks
        #   scalar (ACT-hwdge): wq, wv, wo
        #   gpsimd (swdge):     tables, cos/sin, wk
        nc.gpsimd.dma_start(out=perm_sb, in_=perm[:, :])
        nc.gpsimd.dma_start(out=mask_sb, in_=maskM[:, :])
        nc.gpsimd.dma_start(out=id_sb, in_=id128[:, :])
        for kc in range(_NKC):
            nc.scalar.dma_start(out=wq_sb[:, kc, :], in_=wq_r[:, kc, :])
            nc.gpsimd.dma_start(out=wk_sb[:, kc, :], in_=wk_r[:, kc, :])
        nc.gpsimd.dma_start(out=cs_sb[0:64, :], in_=cs2[:, :])
        nc.gpsimd.dma_start(out=sn_sb[0:64, :], in_=sn2[:, :])
        nc.gpsimd.dma_start(out=cs_sb[64:128, :], in_=cs_sb[0:64, :])
        nc.gpsimd.dma_start(out=sn_sb[64:128, :], in_=sn_sb[0:64, :])
        for kc in range(_NKC):
            nc.scalar.dma_start(out=wv_sb[:, kc, :], in_=wv_r[:, kc, :])
        nc.scalar.dma_start(out=wo_sb,
                            in_=wo.rearrange("(c p) d -> p c d", p=128))
        xts = {}

        def load_xt(lc):
            xt = xtp.tile([128, _NKC, _LC], bf16, tag="xt", name=f"xt{lc}")
            for kc in range(_NKC):
                nc.sync.dma_start(
                    out=xt[:, kc, :],
                    in_=xT_r[:, kc, lc * _LC:(lc + 1) * _LC])
            xts[lc] = xt

        for lc in range(4):
            load_xt(lc)

        # --- work thunks ---
        def proj_qk_thunk(lc, w_sb, dst, c):
            def t():
                ls = slice(lc * _LC, (lc + 1) * _LC)
                xt = xts[lc]
                ps = scr.tile([128, _LC], f32, tag="scr",
                              name=f"ps{lc}_{c}")
                for kc in range(_NKC):
                    nc.tensor.matmul(
                        ps, w_sb[:, kc, c * 128:(c + 1) * 128],
                        xt[:, kc, :],
                        start=(kc == 0), stop=(kc == _NKC - 1))
                nc.vector.tensor_copy(dst[:, c, ls], ps)
            return t

        def rope_thunk(lc, dst, c):
            def t():
                ls = slice(lc * _LC, (lc + 1) * _LC)
                rp = scr.tile([128, _LC], f32, tag="scr",
                              name=f"rp{lc}_{c}")
                nc.tensor.matmul(rp, perm_sb[:, :], dst[:, c, ls],
                                 start=True, stop=True)
                tmp = rtmp.tile([128, _LC], f32, tag="rt")
                nc.vector.tensor_mul(tmp, rp, sn_sb[:, ls])
                nc.vector.tensor_mul(dst[:, c, ls], dst[:, c, ls],
                                     cs_sb[:, ls])
                nc.vector.tensor_add(dst[:, c, ls], dst[:, c, ls], tmp)
            return t

        def proj_v_thunk(lc, j):
            def t():
                xt = xts[lc]
                lt = lc * (_LC // 128) + j
                pv = scr.tile([128, _EG], f32, tag="scr", name=f"pv{lt}")
                for kc in range(_NKC):
                    nc.tensor.matmul(
                        pv, xt[:, kc, j * 128:(j + 1) * 128],
                        wv_sb[:, kc, :],
                        start=(kc == 0), stop=(kc == _NKC - 1))
                nc.vector.tensor_copy(
                    v_sb[:, lt, :, :_HD],
                    pv.rearrange("p (h e) -> p h e", h=_HPG))
            return t

        def norm_thunk(st, h):
            def t():
                qc, qs = st["qc"], st["qs"]
                c, pb = h // 2, 64 * (h % 2)
                drow, otc = st["drow"][h], st["otcs"][h]
                # rank-1 broadcast: ones[1,64].T @ recip_row -> [64, 512]
                bc = scr.tile([128, _QC], f32, tag="scr",
                              name=f"bc{qc}_{h}")
                nc.tensor.matmul(
                    bc[0:64, :], ones_sb[64:65, :], drow[64:65, :],
                    start=True, stop=True)
                otn = otcp.tile([64, _QC], bf16, tag="otn", bufs=3,
                                name=f"otn{qc}_{h}")
                nc.vector.tensor_mul(otn, otc[0:64, :], bc[0:64, :])
                # place normalized O^T at this head's partitions (DMA can
                # cross partition bases; compute engines cannot)
                nc.gpsimd.dma_start(out=oT_sb[pb:pb + 64, c, qs], in_=otn)
            return t

        def oproj_thunk(st, j):
            def t():
                qc = st["qc"]
                lt = qc * (_QC // 128) + j
                ob = otcp.tile([128, _D], f32, tag="ob", bufs=2,
                               name=f"ob{qc}_{j}")
                for n in range(2):
                    op = scr.tile([128, 512], f32, tag="scr",
                                  name=f"op{qc}_{j}_{n}")
                    for cc in range(2):
                        nc.tensor.matmul(
                            op, oT_sb[:, cc, lt * 128:(lt + 1) * 128],
                            wo_sb[:, cc, n * 512:(n + 1) * 512],
                            start=(cc == 0), stop=(cc == 1))
                    nc.vector.tensor_copy(ob[:, n * 512:(n + 1) * 512], op)
                nc.sync.dma_start(
                    out=y[lt * 128:(lt + 1) * 128, :], in_=ob)
            return t

        # --- filler machinery ---
        state = {"fillers": [], "fi": 0}

        def pop_filler(n=1):
            for _ in range(n):
                if state["fi"] < len(state["fillers"]):
                    state["fillers"][state["fi"]]()
                    state["fi"] += 1

        def drain_fillers():
            pop_filler(len(state["fillers"]) - state["fi"])

        # chunk 0 projections emitted directly
        for c in range(2):
            proj_qk_thunk(0, wq_sb, qT_sb, c)()
            proj_qk_thunk(0, wk_sb, kT_sb, c)()
        for dst in (qT_sb, kT_sb):
            for c in range(2):
                rope_thunk(0, dst, c)()
        for j in range(_LC // 128):
            proj_v_thunk(0, j)()

        prev = None
        for qc in range(_NQC):
            drain_fillers()
            # build filler list: prev-chunk norm, next-chunk projections,
            # prev-chunk output projection (emission order = PE order)
            fl = []
            if qc + 1 < _NQC:
                for c in range(2):
                    fl.append(proj_qk_thunk(qc + 1, wq_sb, qT_sb, c))
                    fl.append(proj_qk_thunk(qc + 1, wk_sb, kT_sb, c))
                for dst in (qT_sb, kT_sb):
                    for c in range(2):
                        fl.append(rope_thunk(qc + 1, dst, c))
                for j in range(_LC // 128):
                    fl.append(proj_v_thunk(qc + 1, j))
            if prev is not None:
                for h in range(_HPG):
                    fl.append(norm_thunk(prev, h))
                for j in range(_QC // 128):
                    fl.append(oproj_thunk(prev, j))
            state["fillers"] = fl
            state["fi"] = 0

            # ---- attention for q-chunk qc ----
            q0 = qc * _QC
            qs = slice(q0, q0 + _QC)
            nkt = (qc + 1) * (_QC // 128)
            otcs = []
            drows = []
            for h in range(_HPG):
                c, pb = h // 2, 64 * (h % 2)
                ot = ops.tile([_HD + 1, _QC], f32, tag="ot")
                ngr = (nkt + _GK - 1) // _GK
                for g in range(ngr):
                    kts = list(range(g * _GK, min((g + 1) * _GK, nkt)))
                    sp = sps.tile([128, _GK * _QC], f32, tag="sp")
                    # q columns < dj*128 of a diagonal k-tile are entirely
                    # in the causal-masked region: skip them in scores,
                    # exp and AV (triangular decomposition)
                    for i, kt in enumerate(kts):
                        dj = kt - qc * (_QC // 128)
                        lo = max(dj, 0) * 128
                        nc.tensor.matmul(
                            sp[:, i * _QC + lo:(i + 1) * _QC],
                            kT_sb[pb:pb + 64, c, kt * 128:(kt + 1) * 128],
                            qT_sb[pb:pb + 64, c, q0 + lo:q0 + _QC],
                            start=True, stop=(dj < 0),
                            skip_group_check=True)
                        if dj >= 0:
                            # causal mask: accumulate -1e5 upper-tri into
                            # the diagonal 128-col slice of this k-tile
                            nc.tensor.matmul(
                                sp[:, i * _QC + lo:i * _QC + lo + 128],
                                mask_sb[:, :], id_sb[:, :],
                                start=False, stop=True,
                                skip_group_check=True)
                    pt = ptp.tile([128, _GK * _QC], bf16, tag="pt")
                    diag = any(kt - qc * (_QC // 128) >= 0 for kt in kts)
                    if not diag:
                        na = len(kts) * _QC
                        nc.scalar.activation(pt[:, :na], sp[:, :na], EXP,
                                             scale=0.125)
                    else:
                        # ragged tile starts: exp per tile's written span
                        for i, kt in enumerate(kts):
                            lo = max(kt - qc * (_QC // 128), 0) * 128
                            nc.scalar.activation(
                                pt[:, i * _QC + lo:(i + 1) * _QC],
                                sp[:, i * _QC + lo:(i + 1) * _QC], EXP,
                                scale=0.125)
                    for i, kt in enumerate(kts):
                        lo = max(kt - qc * (_QC // 128), 0) * 128
                        nc.tensor.matmul(
                            ot[:, lo:], v_sb[:, kt, h, :_HD + 1],
                            pt[:, i * _QC + lo:(i + 1) * _QC],
                            start=(kt == 0), stop=(kt == nkt - 1),
                            skip_group_check=True)
                    pop_filler()
                # per-head normalization dance: copy numerator to SBUF,
                # stash the denominator row transposed to [128, 4] so the
                # reciprocal is partition-parallel, restore to a row
                otc = otcp.tile([_HD + 1, _QC], f32, tag="otc", bufs=6,
                                name=f"otc{qc}_{h}")
                nc.scalar.copy(otc, ot[:, :])
                dsb = nrm.tile([128, 4], f32, tag="dsb")
                nc.gpsimd.dma_start(out=dsb, in_=otc[64:65, :])
                drec = nrm.tile([128, 4], f32r, tag="drec")
                with nc.allow_low_precision(reason="recip feeds tf32 matmul"):
                    nc.vector.reciprocal(drec, dsb)
                drow = nrm.tile([65, _QC], f32r, tag="drow", bufs=6,
                                name=f"drow{qc}_{h}")
                nc.gpsimd.dma_start(out=drow[64:65, :], in_=drec)
                otcs.append(otc)
                drows.append(drow)
                pop_filler()
            prev = {"qc": qc, "qs": qs, "otcs": otcs, "drow": drows}
        # tail: last chunk's normalization + output projection
        drain_fillers()
        for h in range(_HPG):
            norm_thunk(prev, h)()
        for j in range(_QC // 128):
            oproj_thunk(prev, j)()
    nc.compile()
    return nc


def get_nc():
    if "nc" not in _CACHE:
        _CACHE["nc"] = _build_nc()
    return _CACHE["nc"]


def make_in_maps(x, token_positions, Q, K, V, O_w):
    """Host-side sharding: per-core input dict (core = b*4 + hg)."""
    import ml_dtypes
    bf16 = ml_dtypes.bfloat16
    x = np.asarray(x, dtype=np.float32)
    tp = np.asarray(token_positions)
    Q = np.asarray(Q, dtype=np.float32)
    K = np.asarray(K, dtype=np.float32)
    V = np.asarray(V, dtype=np.float32)
    O_w = np.asarray(O_w, dtype=np.float32)

    # RoPE tables, [64, L]: rows = head-local e (cos/sin repeated pairwise);
    # the device duplicates to partitions 64..127 (two heads per tile).
    i = np.arange(_HD // 2, dtype=np.float64)
    denom = _THETA ** (2.0 * i / _HD)                      # [32]
    ang = tp.astype(np.float64)[None, :] / denom[:, None]  # [32, L]
    cs2 = np.repeat(np.cos(ang), 2, axis=0).astype(np.float32)
    sn2 = np.repeat(np.sin(ang), 2, axis=0).astype(np.float32)

    # pairwise-rotation permutation (rot(x)[2i] = -x[2i+1], rot[2i+1] = x[2i])
    # as a stationary operand: out = permT.T @ x^T = Perm @ x^T
    p64 = np.zeros((64, 64), np.float32)
    for j in range(_HD // 2):
        p64[2 * j + 1, 2 * j] = -1.0
        p64[2 * j, 2 * j + 1] = 1.0
    permT = np.zeros((128, 128), np.float32)
    permT[0:64, 0:64] = p64
    permT[64:128, 64:128] = p64

    # causal mask as an additive stationary operand: matmul adds
    # maskM.T (-1e5 where q' < k) into the diagonal score tile
    a = np.arange(128)
    maskM = np.where(a[:, None] < a[None, :], -1.0e5, 0.0).astype(np.float32)
    id128 = np.eye(128, dtype=np.float32)

    Qr = Q.reshape(_H, _HD, _D)
    Kr = K.reshape(_H, _HD, _D)
    Vr = V.reshape(_H, _HD, _D)

    in_maps = []
    xT = [np.ascontiguousarray(x[b].T).astype(bf16) for b in range(_B)]
    for core in range(_NCORES):
        b, hg = core // 4, core % 4
        hs = slice(hg * _HPG, (hg + 1) * _HPG)
        in_maps.append({
            "xT": xT[b],
            "wq": Qr[hs].reshape(_EG, _D).T.astype(bf16),
            "wk": Kr[hs].reshape(_EG, _D).T.astype(bf16),
            "wv": Vr[hs].reshape(_EG, _D).T.astype(bf16),
            "wo": O_w[:, hg * _EG:(hg + 1) * _EG].T.astype(bf16),
            "cs2": cs2, "sn2": sn2, "perm": permT,
            "maskM": _tf32(maskM), "id128": id128,
        })
    return in_maps


def run_on_hw(in_maps, trace=False, **kw):
    from concourse.bass_utils import run_bass_kernel_spmd
    nc = get_nc()
    return run_bass_kernel_spmd(nc, in_maps, core_ids=list(range(_NCORES)),
                                trace=trace, **kw)


def kernel(x, token_positions, Q, K, V, O_w):
    in_maps = make_in_maps(x, token_positions, Q, K, V, O_w)
    res = run_on_hw(in_maps)
    out = np.zeros((_B, _L, _D), dtype=np.float32)
    for core in range(_NCORES):
        out[core // 4] += res.results[core]["y"]
    return out
